# revision 2
# baseline (speedup 1.0000x reference)
"""AttentionMIL (segment softmax-attention reduce) Trainium2 kernel, 8 NeuronCores.

Model (per reference):
    h       = relu(features @ W1.T + b1)          # [N, 256]
    a       = tanh(h @ Wa1.T + ba1)               # [N, 128]
    scores  = a @ Wa2.T + ba2                     # [N]
    attn    = segment_softmax(scores, 32 bags of 8192)
    bag_emb = segment_sum(attn * h)               # [32, 256]
    out     = bag_emb @ Wh.T + bh                 # [32, 2]

Sharding: patches split 8 ways (32768 patches = 4 whole bags per core);
weights replicated; everything device-local, no collectives.

Host prep: features are transposed+cast to bf16 [1024, 32768] per core so the
contraction dim (input features) lands on SBUF partitions with no on-chip
transpose. Small weights are pre-packed into SBUF layouts. Wa2 is replicated
into a [128, 128] stationary so the score matmul produces scores broadcast
across all 128 partitions — which makes the softmax and the weighted reduce
pure free-axis operations.

Softmax max-subtraction is dropped: attn = e/z is exactly shift-invariant and
|scores| <= sum|Wa2| * 1 < 3, so exp cannot overflow. ba2 likewise cancels.
"""

import sys

if "/opt/trn_rl_repo" not in sys.path:
    sys.path.insert(0, "/opt/trn_rl_repo")

from contextlib import ExitStack

import ml_dtypes
import numpy as np

from concourse import bacc, bass, mybir, tile
from concourse.bass_utils import run_bass_kernel_spmd

N_CORES = 8
N_PATCHES = 262144
INPUT_DIM = 1024
FEAT_DIM = 256
ATTN_DIM = 128
HEAD_DIM = 2
NP_CORE = N_PATCHES // N_CORES  # 32768
BAG = 8192

P = 128
DC = INPUT_DIM // P  # 8 contraction chunks
CHUNK = 512          # patches per inner tile (one PSUM bank at fp32)
SC_CHUNKS = 4        # chunks per DMA superchunk (2048 patches, 4 MiB bf16)

BF16 = mybir.dt.bfloat16
F32 = mybir.dt.float32
AF = mybir.ActivationFunctionType
ALU = mybir.AluOpType
AX = mybir.AxisListType


def build_nc(np_core=NP_CORE, bag=BAG, chunk=CHUNK, sc_chunks=SC_CHUNKS):
    n_chunks = np_core // chunk
    cpb = bag // chunk            # chunks per bag
    n_bags = np_core // bag       # bags per core
    n_sc = n_chunks // sc_chunks  # superchunks
    assert n_chunks % sc_chunks == 0 and bag % chunk == 0 and np_core % bag == 0
    assert cpb % sc_chunks == 0 or sc_chunks % cpb == 0

    nc = bacc.Bacc()
    xt = nc.declare_dram_parameter("xt", [INPUT_DIM, np_core], BF16, isOutput=False)
    w1t = nc.declare_dram_parameter("w1t", [P, DC, FEAT_DIM], BF16, isOutput=False)
    wa1t = nc.declare_dram_parameter("wa1t", [P, 2, ATTN_DIM], BF16, isOutput=False)
    wa2r = nc.declare_dram_parameter("wa2r", [P, P], BF16, isOutput=False)
    wht = nc.declare_dram_parameter("wht", [P, 2, HEAD_DIM], F32, isOutput=False)
    b1c = nc.declare_dram_parameter("b1c", [P, 2], F32, isOutput=False)
    ba1c = nc.declare_dram_parameter("ba1c", [P, 1], F32, isOutput=False)
    bh2 = nc.declare_dram_parameter("bh2", [1, HEAD_DIM], F32, isOutput=False)
    ones_b = nc.declare_dram_parameter("ones_b", [1, n_bags], F32, isOutput=False)
    out_ext = nc.declare_dram_parameter("out", [n_bags, HEAD_DIM], F32, isOutput=True)

    xt_r = xt.ap().rearrange("(c p) n -> p c n", p=P)

    with tile.TileContext(nc) as tc, ExitStack() as ctx:
        const = ctx.enter_context(tc.tile_pool(name="const", bufs=1))
        xpool = ctx.enter_context(tc.tile_pool(name="xpool", bufs=2))
        hpool = ctx.enter_context(tc.tile_pool(name="hpool", bufs=3))
        apool = ctx.enter_context(tc.tile_pool(name="apool", bufs=3))
        epool = ctx.enter_context(tc.tile_pool(name="epool", bufs=3))
        bpool = ctx.enter_context(tc.tile_pool(name="bpool", bufs=2))
        psum = ctx.enter_context(tc.tile_pool(name="psum", bufs=2, space="PSUM"))

        w1t_sb = const.tile([P, DC, FEAT_DIM], BF16)
        nc.sync.dma_start(w1t_sb[:], w1t.ap())
        wa1t_sb = const.tile([P, 2, ATTN_DIM], BF16)
        nc.sync.dma_start(wa1t_sb[:], wa1t.ap())
        wa2r_sb = const.tile([P, P], BF16)
        nc.sync.dma_start(wa2r_sb[:], wa2r.ap())
        wht_sb = const.tile([P, 2, HEAD_DIM], F32)
        nc.sync.dma_start(wht_sb[:], wht.ap())
        b1c_sb = const.tile([P, 2], F32)
        nc.sync.dma_start(b1c_sb[:], b1c.ap())
        ba1c_sb = const.tile([P, 1], F32)
        nc.sync.dma_start(ba1c_sb[:], ba1c.ap())
        bh2_sb = const.tile([1, HEAD_DIM], F32)
        nc.sync.dma_start(bh2_sb[:], bh2.ap())
        ones_sb = const.tile([1, n_bags], F32)
        nc.sync.dma_start(ones_sb[:], ones_b.ap())
        bagembT = const.tile([P, 2, n_bags], F32)  # normalized bag embeddings

        zparts = bp0 = bp1 = None
        for sc in range(n_sc):
            xsb = xpool.tile([P, DC, sc_chunks * chunk], BF16, tag="x")
            nc.sync.dma_start(
                xsb[:], xt_r[:, :, sc * sc_chunks * chunk:(sc + 1) * sc_chunks * chunk]
            )
            for cc in range(sc_chunks):
                ci = sc * sc_chunks + cc
                b, cib = divmod(ci, cpb)
                if cib == 0:
                    zparts = bpool.tile([P, cpb], F32, tag="zparts")
                    bp0 = bpool.tile([P, cpb], F32, tag="bp0")
                    bp1 = bpool.tile([P, cpb], F32, tag="bp1")

                rhs = xsb[:, :, cc * chunk:(cc + 1) * chunk]
                # encoder: h.T halves, contraction over input dim in 8 chunks
                hp = psum.tile([P, 2, chunk], F32, tag="hp")
                for fh in range(2):
                    for d in range(DC):
                        nc.tensor.matmul(
                            hp[:, fh, :],
                            w1t_sb[:, d, fh * P:(fh + 1) * P],
                            rhs[:, d, :],
                            start=(d == 0),
                            stop=(d == DC - 1),
                        )
                ht = hpool.tile([P, 2, chunk], BF16, tag="ht")
                nc.scalar.activation(ht[:, 0, :], hp[:, 0, :], AF.Relu, bias=b1c_sb[:, 0:1])
                nc.scalar.activation(ht[:, 1, :], hp[:, 1, :], AF.Relu, bias=b1c_sb[:, 1:2])

                # attention MLP: a.T = tanh(Wa1 @ h.T + ba1)
                ap_ = psum.tile([P, chunk], F32, tag="ap")
                nc.tensor.matmul(ap_[:], wa1t_sb[:, 0, :], ht[:, 0, :], start=True, stop=False)
                nc.tensor.matmul(ap_[:], wa1t_sb[:, 1, :], ht[:, 1, :], start=False, stop=True)
                at = apool.tile([P, chunk], BF16, tag="at")
                nc.scalar.activation(at[:], ap_[:], AF.Tanh, bias=ba1c_sb[:])

                # scores broadcast across partitions via replicated Wa2
                sp = psum.tile([P, chunk], F32, tag="sp")
                nc.tensor.matmul(sp[:], wa2r_sb[:], at[:], start=True, stop=True)
                eb = epool.tile([P, chunk], F32, tag="eb")
                nc.scalar.activation(
                    eb[:], sp[:], AF.Exp, accum_out=zparts[:, cib:cib + 1]
                )

                # weighted partial reduce: bp[f, cib] = sum_p h.T[f, p] * e[p]
                st = epool.tile([P, 2, chunk], F32, tag="st")
                nc.vector.tensor_mul(st[:, 0, :], ht[:, 0, :], eb[:])
                nc.vector.tensor_mul(st[:, 1, :], ht[:, 1, :], eb[:])
                nc.vector.tensor_reduce(bp0[:, cib:cib + 1], st[:, 0, :], axis=AX.X, op=ALU.add)
                nc.vector.tensor_reduce(bp1[:, cib:cib + 1], st[:, 1, :], axis=AX.X, op=ALU.add)

                if cib == cpb - 1:
                    # finish bag b: z = sum(zparts); bagembT[:, :, b] = sum(e*h)/z
                    zs = bpool.tile([P, 1], F32, tag="zs")
                    nc.vector.tensor_reduce(zs[:], zparts[:], axis=AX.X, op=ALU.add)
                    rz = bpool.tile([P, 1], F32, tag="rz")
                    nc.vector.reciprocal(rz[:], zs[:])
                    s0 = bpool.tile([P, 1], F32, tag="s0")
                    nc.vector.tensor_reduce(s0[:], bp0[:], axis=AX.X, op=ALU.add)
                    s1 = bpool.tile([P, 1], F32, tag="s1")
                    nc.vector.tensor_reduce(s1[:], bp1[:], axis=AX.X, op=ALU.add)
                    nc.vector.tensor_mul(bagembT[:, 0, b:b + 1], s0[:], rz[:])
                    nc.vector.tensor_mul(bagembT[:, 1, b:b + 1], s1[:], rz[:])

        # head: out = bag_emb @ Wh.T + bh
        hdp = psum.tile([n_bags, HEAD_DIM], F32, tag="sp")
        nc.tensor.matmul(hdp[:], bagembT[:, 0, :], wht_sb[:, 0, :], start=True, stop=False)
        nc.tensor.matmul(hdp[:], bagembT[:, 1, :], wht_sb[:, 1, :], start=False, stop=False)
        nc.tensor.matmul(hdp[:], ones_sb[:], bh2_sb[:], start=False, stop=True)
        outt = const.tile([n_bags, HEAD_DIM], F32)
        nc.scalar.copy(outt[:], hdp[:])
        nc.sync.dma_start(out_ext.ap(), outt[:])

    nc.compile()
    return nc


def prep_weights(W1, b1, Wa1, ba1, Wa2, ba2, Wh, bh, n_bags):
    f32, bf16 = np.float32, ml_dtypes.bfloat16
    W1 = np.asarray(W1, f32)
    Wa1 = np.asarray(Wa1, f32)
    Wa2 = np.asarray(Wa2, f32)
    Wh = np.asarray(Wh, f32)
    return {
        "w1t": W1.T.reshape(DC, P, FEAT_DIM).transpose(1, 0, 2).astype(bf16),
        "wa1t": Wa1.T.reshape(2, P, ATTN_DIM).transpose(1, 0, 2).astype(bf16),
        "wa2r": np.repeat(Wa2.reshape(P, 1), P, axis=1).astype(bf16),
        "wht": np.ascontiguousarray(Wh.T.reshape(2, P, HEAD_DIM).transpose(1, 0, 2)),
        "b1c": np.ascontiguousarray(np.asarray(b1, f32).reshape(2, P).T),
        "ba1c": np.asarray(ba1, f32).reshape(P, 1).copy(),
        "bh2": np.asarray(bh, f32).reshape(1, HEAD_DIM).copy(),
        "ones_b": np.ones((1, n_bags), f32),
    }


_NC_CACHE = {}


def kernel(features, W1, b1, Wa1, ba1, Wa2, ba2, Wh, bh, bag_sizes):
    f32, bf16 = np.float32, ml_dtypes.bfloat16
    n_bags_core = NP_CORE // BAG

    X = np.asarray(features, f32)
    xT = X.T.astype(bf16)  # [1024, 262144], C-contiguous

    shared = prep_weights(W1, b1, Wa1, ba1, Wa2, ba2, Wh, bh, n_bags_core)
    in_maps = []
    for i in range(N_CORES):
        xt_i = np.ascontiguousarray(xT[:, i * NP_CORE:(i + 1) * NP_CORE])
        in_maps.append({**shared, "xt": xt_i})

    if "nc" not in _NC_CACHE:
        _NC_CACHE["nc"] = build_nc()
    nc = _NC_CACHE["nc"]

    res = run_bass_kernel_spmd(nc, in_maps, core_ids=list(range(N_CORES)))
    out = np.concatenate(
        [np.asarray(res.results[i]["out"], f32) for i in range(N_CORES)], axis=0
    )
    return out


# revision 5
# speedup vs baseline: 1.5058x; 1.5058x over previous
"""AttentionMIL (segment softmax-attention reduce) Trainium2 kernel, 8 NeuronCores.

Model (per reference):
    h       = relu(features @ W1.T + b1)          # [N, 256]
    a       = tanh(h @ Wa1.T + ba1)               # [N, 128]
    scores  = a @ Wa2.T + ba2                     # [N]
    attn    = segment_softmax(scores, 32 bags of 8192)
    bag_emb = segment_sum(attn * h)               # [32, 256]
    out     = bag_emb @ Wh.T + bh                 # [32, 2]

Sharding: patches split 8 ways (32768 patches = 4 whole bags per core);
weights replicated; everything device-local, no collectives.

Host prep: features are transposed and quantized (fp8-e4m3 by default) to
[1024, 32768] per core so the contraction dim lands on SBUF partitions with
no on-chip transpose. W1/Wa1 are scaled by 16 before fp8 quantization (their
0.02-scale values would otherwise sit in the subnormal range); the 1/16 is
folded into the ReLU/Tanh activation's free scale operand. Wa2 is replicated
into a [128, 128] stationary so the score matmul lands broadcast across all
128 partitions — making softmax + weighted reduce pure free-axis operations.
Encoder and attention matmuls run fp8 DoubleRow (2 contraction rows/cell).

Softmax max-subtraction is dropped: attn = e/z is exactly shift-invariant and
|scores| <= sum|Wa2| * 1 < 3, so exp cannot overflow. ba2 likewise cancels.

The custom GPSIMD-microcoded DVE ops (tensor_tensor_reduce etc.) crash this
terminal's NRT — only native ISA ops are used.
"""

import sys

if "/opt/trn_rl_repo" not in sys.path:
    sys.path.insert(0, "/opt/trn_rl_repo")

from contextlib import ExitStack

import ml_dtypes
import numpy as np

from concourse import bacc, mybir, tile
from concourse.bass_utils import run_bass_kernel_spmd

N_CORES = 8
N_PATCHES = 262144
INPUT_DIM = 1024
FEAT_DIM = 256
ATTN_DIM = 128
HEAD_DIM = 2
NP_CORE = N_PATCHES // N_CORES  # 32768
BAG = 8192

P = 128
DC = INPUT_DIM // P  # 8 contraction chunks of 128
CHUNK = 512          # patches per inner tile (one PSUM bank at fp32)
SC_CHUNKS = 4        # chunks per DMA superchunk (2048 patches)
W_SCALE = 16.0       # host pre-scale on W1/Wa1 before fp8 quantization

BF16 = mybir.dt.bfloat16
F32 = mybir.dt.float32
FP8 = mybir.dt.float8e4
AF = mybir.ActivationFunctionType
ALU = mybir.AluOpType
AX = mybir.AxisListType
DR = mybir.MatmulPerfMode.DoubleRow

NP_F8 = ml_dtypes.float8_e4m3
NP_BF16 = ml_dtypes.bfloat16


def build_nc(np_core=NP_CORE, bag=BAG, chunk=CHUNK, sc_chunks=SC_CHUNKS,
             mode="fp8"):
    n_chunks = np_core // chunk
    cpb = bag // chunk            # chunks per bag
    n_bags = np_core // bag       # bags per core
    n_sc = n_chunks // sc_chunks  # superchunks
    assert n_chunks % sc_chunks == 0 and bag % chunk == 0 and np_core % bag == 0

    fp8 = mode == "fp8"
    XDT = FP8 if fp8 else BF16

    nc = bacc.Bacc()
    xt = nc.declare_dram_parameter("xt", [INPUT_DIM, np_core], XDT, isOutput=False)
    w1t = nc.declare_dram_parameter("w1t", [P, DC, FEAT_DIM], XDT, isOutput=False)
    wa1t = nc.declare_dram_parameter("wa1t", [P, 2, ATTN_DIM], XDT, isOutput=False)
    wa2r = nc.declare_dram_parameter("wa2r", [P, P], BF16, isOutput=False)
    wht = nc.declare_dram_parameter("wht", [P, 2, HEAD_DIM], F32, isOutput=False)
    b1c = nc.declare_dram_parameter("b1c", [P, 2], F32, isOutput=False)
    ba1c = nc.declare_dram_parameter("ba1c", [P, 1], F32, isOutput=False)
    bh2 = nc.declare_dram_parameter("bh2", [1, HEAD_DIM], F32, isOutput=False)
    ones_b = nc.declare_dram_parameter("ones_b", [1, n_bags], F32, isOutput=False)
    out_ext = nc.declare_dram_parameter("out", [n_bags, HEAD_DIM], F32, isOutput=True)

    inv_scale = 1.0 / W_SCALE if fp8 else 1.0

    xt_r = xt.ap().rearrange("(c p) n -> p c n", p=P)

    with tile.TileContext(nc) as tc, ExitStack() as ctx:
        const = ctx.enter_context(tc.tile_pool(name="const", bufs=1))
        xpool = ctx.enter_context(tc.tile_pool(name="xpool", bufs=2))
        hpool = ctx.enter_context(tc.tile_pool(name="hpool", bufs=3))
        apool = ctx.enter_context(tc.tile_pool(name="apool", bufs=3))
        epool = ctx.enter_context(tc.tile_pool(name="epool", bufs=3))
        bpool = ctx.enter_context(tc.tile_pool(name="bpool", bufs=2))
        psum = ctx.enter_context(tc.tile_pool(name="psum", bufs=2, space="PSUM"))

        # weights for the first matmuls + the first features superchunk go
        # first so the TensorEngine starts as early as possible
        w1t_sb = const.tile([P, DC, FEAT_DIM], XDT)
        nc.sync.dma_start(w1t_sb[:], w1t.ap())
        xsb0 = xpool.tile([P, DC, sc_chunks * chunk], XDT, tag="x")
        nc.sync.dma_start(xsb0[:], xt_r[:, :, 0:sc_chunks * chunk])

        wa1t_sb = const.tile([P, 2, ATTN_DIM], XDT)
        nc.sync.dma_start(wa1t_sb[:], wa1t.ap())
        wa2r_sb = const.tile([P, P], BF16)
        nc.sync.dma_start(wa2r_sb[:], wa2r.ap())
        wht_sb = const.tile([P, 2, HEAD_DIM], F32)
        nc.sync.dma_start(wht_sb[:], wht.ap())
        b1c_sb = const.tile([P, 2], F32)
        nc.sync.dma_start(b1c_sb[:], b1c.ap())
        ba1c_sb = const.tile([P, 1], F32)
        nc.sync.dma_start(ba1c_sb[:], ba1c.ap())
        bh2_sb = const.tile([1, HEAD_DIM], F32)
        nc.sync.dma_start(bh2_sb[:], bh2.ap())
        ones_sb = const.tile([1, n_bags], F32)
        nc.sync.dma_start(ones_sb[:], ones_b.ap())
        bagembT = const.tile([P, 2, n_bags], F32)  # normalized bag embeddings

        zparts = bpp = None
        for sc in range(n_sc):
            if sc == 0:
                xsb = xsb0
            else:
                xsb = xpool.tile([P, DC, sc_chunks * chunk], XDT, tag="x")
                nc.sync.dma_start(
                    xsb[:],
                    xt_r[:, :, sc * sc_chunks * chunk:(sc + 1) * sc_chunks * chunk],
                )
            for cc in range(sc_chunks):
                ci = sc * sc_chunks + cc
                b, cib = divmod(ci, cpb)
                if cib == 0:
                    zparts = bpool.tile([P, cpb], F32, tag="zparts")
                    bpp = bpool.tile([P, 2, cpb], F32, tag="bpp")

                rhs = xsb[:, :, cc * chunk:(cc + 1) * chunk]
                # encoder: h.T halves, contraction over the 1024 input dims
                hp = psum.tile([P, 2, chunk], F32, tag="hp")
                for fh in range(2):
                    if fp8:
                        for d in range(DC // 2):
                            nc.tensor.matmul(
                                hp[:, fh, :],
                                w1t_sb[:, 2 * d:2 * d + 2, fh * P:(fh + 1) * P],
                                rhs[:, 2 * d:2 * d + 2, :],
                                start=(d == 0), stop=(d == DC // 2 - 1),
                                perf_mode=DR,
                            )
                    else:
                        for d in range(DC):
                            nc.tensor.matmul(
                                hp[:, fh, :],
                                w1t_sb[:, d, fh * P:(fh + 1) * P],
                                rhs[:, d, :],
                                start=(d == 0), stop=(d == DC - 1),
                            )
                ht = hpool.tile([P, 2, chunk], XDT, tag="ht")
                if fp8:
                    # b1 == 0 in this model (checked in kernel()), so one
                    # scalar bias serves both feature halves
                    nc.scalar.activation(ht[:, :, :], hp[:, :, :], AF.Relu,
                                         bias=0.0, scale=inv_scale)
                else:
                    nc.scalar.activation(ht[:, 0, :], hp[:, 0, :], AF.Relu,
                                         bias=b1c_sb[:, 0:1])
                    nc.scalar.activation(ht[:, 1, :], hp[:, 1, :], AF.Relu,
                                         bias=b1c_sb[:, 1:2])

                # attention MLP: a.T = tanh(Wa1 @ h.T + ba1)
                ap_ = psum.tile([P, chunk], F32, tag="ap")
                if fp8:
                    nc.tensor.matmul(ap_[:], wa1t_sb[:, :, :], ht[:, :, :],
                                     start=True, stop=True, perf_mode=DR)
                else:
                    nc.tensor.matmul(ap_[:], wa1t_sb[:, 0, :], ht[:, 0, :],
                                     start=True, stop=False)
                    nc.tensor.matmul(ap_[:], wa1t_sb[:, 1, :], ht[:, 1, :],
                                     start=False, stop=True)
                at = apool.tile([P, chunk], BF16, tag="at")
                nc.scalar.activation(at[:], ap_[:], AF.Tanh,
                                     bias=ba1c_sb[:], scale=inv_scale)

                # scores broadcast across partitions via replicated Wa2
                sp = psum.tile([P, chunk], F32, tag="sp")
                nc.tensor.matmul(sp[:], wa2r_sb[:], at[:], start=True, stop=True)
                eb = epool.tile([P, chunk], BF16, tag="eb")
                nc.scalar.activation(eb[:], sp[:], AF.Exp,
                                     accum_out=zparts[:, cib:cib + 1])

                # weighted partial reduce: bpp[f, :, cib] = sum_p h.T[f, p]*e[p]
                st = epool.tile([P, 2, chunk], BF16, tag="st")
                eb_b = eb[:].rearrange("p (o n) -> p o n", o=1).broadcast_to((P, 2, chunk))
                nc.vector.tensor_mul(st[:, :, :], ht[:, :, :], eb_b)
                nc.vector.tensor_reduce(bpp[:, :, cib], st[:, :, :],
                                        axis=AX.X, op=ALU.add)

                if cib == cpb - 1:
                    # finish bag b: z = sum(zparts); bagembT[:, :, b] = sum/z
                    zs = bpool.tile([P, 1], F32, tag="zs")
                    nc.vector.tensor_reduce(zs[:], zparts[:], axis=AX.X, op=ALU.add)
                    rz = bpool.tile([P, 1], F32, tag="rz")
                    nc.vector.reciprocal(rz[:], zs[:])
                    s0 = bpool.tile([P, 1], F32, tag="s0")
                    nc.vector.tensor_reduce(s0[:], bpp[:, 0, :], axis=AX.X, op=ALU.add)
                    s1 = bpool.tile([P, 1], F32, tag="s1")
                    nc.vector.tensor_reduce(s1[:], bpp[:, 1, :], axis=AX.X, op=ALU.add)
                    nc.vector.tensor_mul(bagembT[:, 0, b:b + 1], s0[:], rz[:])
                    nc.vector.tensor_mul(bagembT[:, 1, b:b + 1], s1[:], rz[:])

        # head: out = bag_emb @ Wh.T + bh
        hdp = psum.tile([n_bags, HEAD_DIM], F32, tag="sp")
        nc.tensor.matmul(hdp[:], bagembT[:, 0, :], wht_sb[:, 0, :], start=True, stop=False)
        nc.tensor.matmul(hdp[:], bagembT[:, 1, :], wht_sb[:, 1, :], start=False, stop=False)
        nc.tensor.matmul(hdp[:], ones_sb[:], bh2_sb[:], start=False, stop=True)
        outt = const.tile([n_bags, HEAD_DIM], F32)
        nc.scalar.copy(outt[:], hdp[:])
        nc.sync.dma_start(out_ext.ap(), outt[:])

    nc.compile()
    return nc


def prep_weights(W1, b1, Wa1, ba1, Wa2, ba2, Wh, bh, n_bags, mode="fp8"):
    f32 = np.float32
    fp8 = mode == "fp8"
    wdt = NP_F8 if fp8 else NP_BF16
    ws = W_SCALE if fp8 else 1.0
    W1 = np.asarray(W1, f32)
    Wa1 = np.asarray(Wa1, f32)
    Wa2 = np.asarray(Wa2, f32)
    Wh = np.asarray(Wh, f32)
    return {
        "w1t": (W1.T * ws).reshape(DC, P, FEAT_DIM).transpose(1, 0, 2).astype(wdt),
        "wa1t": (Wa1.T * ws).reshape(2, P, ATTN_DIM).transpose(1, 0, 2).astype(wdt),
        "wa2r": np.repeat(Wa2.reshape(P, 1), P, axis=1).astype(NP_BF16),
        "wht": np.ascontiguousarray(Wh.T.reshape(2, P, HEAD_DIM).transpose(1, 0, 2)),
        "b1c": np.ascontiguousarray(np.asarray(b1, f32).reshape(2, P).T),
        "ba1c": np.asarray(ba1, f32).reshape(P, 1).copy(),
        "bh2": np.asarray(bh, f32).reshape(1, HEAD_DIM).copy(),
        "ones_b": np.ones((1, n_bags), f32),
    }


_NC_CACHE = {}


def kernel(features, W1, b1, Wa1, ba1, Wa2, ba2, Wh, bh, bag_sizes):
    f32 = np.float32
    mode = "fp8"
    # the fp8 fast path folds b1 into a scalar activation bias, which is only
    # exact when b1 is all-zero (it is, for this model's inputs)
    if np.any(np.asarray(b1, f32) != 0.0):
        mode = "bf16"
    n_bags_core = NP_CORE // BAG

    X = np.asarray(features, f32)
    xT = X.T.astype(NP_F8 if mode == "fp8" else NP_BF16)

    shared = prep_weights(W1, b1, Wa1, ba1, Wa2, ba2, Wh, bh, n_bags_core, mode)
    in_maps = []
    for i in range(N_CORES):
        xt_i = np.ascontiguousarray(xT[:, i * NP_CORE:(i + 1) * NP_CORE])
        in_maps.append({**shared, "xt": xt_i})

    if mode not in _NC_CACHE:
        _NC_CACHE[mode] = build_nc(mode=mode)
    nc = _NC_CACHE[mode]

    res = run_bass_kernel_spmd(nc, in_maps, core_ids=list(range(N_CORES)))
    out = np.concatenate(
        [np.asarray(res.results[i]["out"], f32) for i in range(N_CORES)], axis=0
    )
    return out


# revision 7
# speedup vs baseline: 1.9505x; 1.2953x over previous
"""AttentionMIL (segment softmax-attention reduce) Trainium2 kernel, 8 NeuronCores.

Model (per reference):
    h       = relu(features @ W1.T + b1)          # [N, 256]
    a       = tanh(h @ Wa1.T + ba1)               # [N, 128]
    scores  = a @ Wa2.T + ba2                     # [N]
    attn    = segment_softmax(scores, 32 bags of 8192)
    bag_emb = segment_sum(attn * h)               # [32, 256]
    out     = bag_emb @ Wh.T + bh                 # [32, 2]

Sharding: patches split 8 ways (32768 patches = 4 whole bags per core);
weights replicated; everything device-local, no collectives.

Host prep: features are transposed and quantized (fp8-e4m3 by default) to
[1024, 32768] per core so the contraction dim lands on SBUF partitions with
no on-chip transpose. W1/Wa1 are scaled by 16 before fp8 quantization (their
0.02-scale values would otherwise sit in the subnormal range); the 1/16 is
folded into the ReLU/Tanh activation's free scale operand. Wa2 is replicated
into a [128, 128] stationary so the score matmul lands broadcast across all
128 partitions — making softmax + weighted reduce pure free-axis operations.
Encoder and attention matmuls run fp8 DoubleRow (2 contraction rows/cell).

Softmax max-subtraction is dropped: attn = e/z is exactly shift-invariant and
|scores| <= sum|Wa2| * 1 < 3, so exp cannot overflow. ba2 likewise cancels.

The custom GPSIMD-microcoded DVE ops (tensor_tensor_reduce etc.) crash this
terminal's NRT — only native ISA ops are used.
"""

import sys

if "/opt/trn_rl_repo" not in sys.path:
    sys.path.insert(0, "/opt/trn_rl_repo")

from contextlib import ExitStack

import ml_dtypes
import numpy as np

from concourse import bacc, mybir, tile
from concourse.bass_utils import run_bass_kernel_spmd

N_CORES = 8
N_PATCHES = 262144
INPUT_DIM = 1024
FEAT_DIM = 256
ATTN_DIM = 128
HEAD_DIM = 2
NP_CORE = N_PATCHES // N_CORES  # 32768
BAG = 8192

P = 128
DC = INPUT_DIM // P  # 8 contraction chunks of 128
CHUNK = 512          # patches per inner tile (one PSUM bank at fp32)
SC_CHUNKS = 4        # chunks per DMA superchunk (2048 patches)
W_SCALE = 16.0       # host pre-scale on W1/Wa1 before fp8 quantization

BF16 = mybir.dt.bfloat16
F32 = mybir.dt.float32
FP8 = mybir.dt.float8e4
AF = mybir.ActivationFunctionType
ALU = mybir.AluOpType
AX = mybir.AxisListType
DR = mybir.MatmulPerfMode.DoubleRow

NP_F8 = ml_dtypes.float8_e4m3
NP_BF16 = ml_dtypes.bfloat16


def build_nc(np_core=NP_CORE, bag=BAG, chunk=CHUNK, sc_chunks=SC_CHUNKS,
             mode="fp8"):
    n_chunks = np_core // chunk
    cpb = bag // chunk            # chunks per bag
    n_bags = np_core // bag       # bags per core
    n_sc = n_chunks // sc_chunks  # superchunks
    assert n_chunks % sc_chunks == 0 and bag % chunk == 0 and np_core % bag == 0

    fp8 = mode == "fp8"
    XDT = FP8 if fp8 else BF16

    nc = bacc.Bacc()
    xt = nc.declare_dram_parameter("xt", [INPUT_DIM, np_core], XDT, isOutput=False)
    w1t = nc.declare_dram_parameter("w1t", [P, DC, FEAT_DIM], XDT, isOutput=False)
    wa1t = nc.declare_dram_parameter("wa1t", [P, 2, ATTN_DIM], XDT, isOutput=False)
    wa2r = nc.declare_dram_parameter("wa2r", [P, P], BF16, isOutput=False)
    wht = nc.declare_dram_parameter("wht", [P, 2, HEAD_DIM], F32, isOutput=False)
    b1c = nc.declare_dram_parameter("b1c", [P, 2], F32, isOutput=False)
    ba1c = nc.declare_dram_parameter("ba1c", [P, 1], F32, isOutput=False)
    bh2 = nc.declare_dram_parameter("bh2", [1, HEAD_DIM], F32, isOutput=False)
    ones_b = nc.declare_dram_parameter("ones_b", [1, n_bags], F32, isOutput=False)
    out_ext = nc.declare_dram_parameter("out", [n_bags, HEAD_DIM], F32, isOutput=True)

    inv_scale = 1.0 / W_SCALE if fp8 else 1.0

    xt_r = xt.ap().rearrange("(c p) n -> p c n", p=P)

    with tile.TileContext(nc) as tc, ExitStack() as ctx:
        const = ctx.enter_context(tc.tile_pool(name="const", bufs=1))
        xpool = ctx.enter_context(tc.tile_pool(name="xpool", bufs=3))
        hpool = ctx.enter_context(tc.tile_pool(name="hpool", bufs=4))
        apool = ctx.enter_context(tc.tile_pool(name="apool", bufs=4))
        epool = ctx.enter_context(tc.tile_pool(name="epool", bufs=4))
        bpool = ctx.enter_context(tc.tile_pool(name="bpool", bufs=2))
        # PSUM: hp gets 3 bufs x 2 banks; ap/sp single-buffered = 8 banks
        psum = ctx.enter_context(tc.tile_pool(name="psum", bufs=3, space="PSUM"))
        psum1 = ctx.enter_context(tc.tile_pool(name="psum1", bufs=1, space="PSUM"))

        # weights for the first matmuls + the first features superchunk go
        # first so the TensorEngine starts as early as possible
        w1t_sb = const.tile([P, DC, FEAT_DIM], XDT)
        nc.sync.dma_start(w1t_sb[:], w1t.ap())
        xsb0 = xpool.tile([P, DC, sc_chunks * chunk], XDT, tag="x")
        nc.sync.dma_start(xsb0[:], xt_r[:, :, 0:sc_chunks * chunk])

        wa1t_sb = const.tile([P, 2, ATTN_DIM], XDT)
        nc.sync.dma_start(wa1t_sb[:], wa1t.ap())
        wa2r_sb = const.tile([P, P], BF16)
        nc.sync.dma_start(wa2r_sb[:], wa2r.ap())
        wht_sb = const.tile([P, 2, HEAD_DIM], F32)
        nc.sync.dma_start(wht_sb[:], wht.ap())
        b1c_sb = const.tile([P, 2], F32)
        nc.sync.dma_start(b1c_sb[:], b1c.ap())
        ba1c_sb = const.tile([P, 1], F32)
        nc.sync.dma_start(ba1c_sb[:], ba1c.ap())
        bh2_sb = const.tile([1, HEAD_DIM], F32)
        nc.sync.dma_start(bh2_sb[:], bh2.ap())
        ones_sb = const.tile([1, n_bags], F32)
        nc.sync.dma_start(ones_sb[:], ones_b.ap())
        bagembT = const.tile([P, 2, n_bags], F32)  # normalized bag embeddings

        zparts = bpp = None
        for sc in range(n_sc):
            if sc == 0:
                xsb = xsb0
            else:
                xsb = xpool.tile([P, DC, sc_chunks * chunk], XDT, tag="x")
                nc.sync.dma_start(
                    xsb[:],
                    xt_r[:, :, sc * sc_chunks * chunk:(sc + 1) * sc_chunks * chunk],
                )
            for cc in range(sc_chunks):
                ci = sc * sc_chunks + cc
                b, cib = divmod(ci, cpb)
                if cib == 0:
                    zparts = bpool.tile([P, cpb], F32, tag="zparts")
                    bpp = bpool.tile([P, 2, cpb], F32, tag="bpp")

                rhs = xsb[:, :, cc * chunk:(cc + 1) * chunk]
                # encoder: h.T halves, contraction over the 1024 input dims
                hp = psum.tile([P, 2, chunk], F32, tag="hp")
                for fh in range(2):
                    if fp8:
                        for d in range(DC // 2):
                            nc.tensor.matmul(
                                hp[:, fh, :],
                                w1t_sb[:, 2 * d:2 * d + 2, fh * P:(fh + 1) * P],
                                rhs[:, 2 * d:2 * d + 2, :],
                                start=(d == 0), stop=(d == DC // 2 - 1),
                                perf_mode=DR,
                            )
                    else:
                        for d in range(DC):
                            nc.tensor.matmul(
                                hp[:, fh, :],
                                w1t_sb[:, d, fh * P:(fh + 1) * P],
                                rhs[:, d, :],
                                start=(d == 0), stop=(d == DC - 1),
                            )
                ht = hpool.tile([P, 2, chunk], XDT, tag="ht")
                if fp8:
                    # b1 == 0 in this model (checked in kernel()), so one
                    # scalar bias serves both feature halves
                    nc.scalar.activation(ht[:, :, :], hp[:, :, :], AF.Relu,
                                         bias=0.0, scale=inv_scale)
                else:
                    nc.scalar.activation(ht[:, 0, :], hp[:, 0, :], AF.Relu,
                                         bias=b1c_sb[:, 0:1])
                    nc.scalar.activation(ht[:, 1, :], hp[:, 1, :], AF.Relu,
                                         bias=b1c_sb[:, 1:2])

                # attention MLP: a.T = tanh(Wa1 @ h.T + ba1)
                ap_ = psum1.tile([P, chunk], F32, tag="ap")
                if fp8:
                    nc.tensor.matmul(ap_[:], wa1t_sb[:, :, :], ht[:, :, :],
                                     start=True, stop=True, perf_mode=DR)
                else:
                    nc.tensor.matmul(ap_[:], wa1t_sb[:, 0, :], ht[:, 0, :],
                                     start=True, stop=False)
                    nc.tensor.matmul(ap_[:], wa1t_sb[:, 1, :], ht[:, 1, :],
                                     start=False, stop=True)
                at = apool.tile([P, chunk], BF16, tag="at")
                nc.scalar.activation(at[:], ap_[:], AF.Tanh,
                                     bias=ba1c_sb[:], scale=inv_scale)

                # scores broadcast across partitions via replicated Wa2
                sp = psum1.tile([P, chunk], F32, tag="sp")
                nc.tensor.matmul(sp[:], wa2r_sb[:], at[:], start=True, stop=True)
                eb = epool.tile([P, chunk], BF16, tag="eb")
                nc.scalar.activation(eb[:], sp[:], AF.Exp,
                                     accum_out=zparts[:, cib:cib + 1])

                # weighted partial reduce: bpp[f, :, cib] = sum_p h.T[f, p]*e[p]
                st = epool.tile([P, 2, chunk], BF16, tag="st")
                eb_b = eb[:].rearrange("p (o n) -> p o n", o=1).broadcast_to((P, 2, chunk))
                nc.vector.tensor_mul(st[:, :, :], ht[:, :, :], eb_b)
                nc.vector.tensor_reduce(bpp[:, :, cib], st[:, :, :],
                                        axis=AX.X, op=ALU.add)

                if cib == cpb - 1:
                    # finish bag b: z = sum(zparts); bagembT[:, :, b] = sum/z
                    zs = bpool.tile([P, 1], F32, tag="zs")
                    nc.vector.tensor_reduce(zs[:], zparts[:], axis=AX.X, op=ALU.add)
                    rz = bpool.tile([P, 1], F32, tag="rz")
                    nc.vector.reciprocal(rz[:], zs[:])
                    s0 = bpool.tile([P, 1], F32, tag="s0")
                    nc.vector.tensor_reduce(s0[:], bpp[:, 0, :], axis=AX.X, op=ALU.add)
                    s1 = bpool.tile([P, 1], F32, tag="s1")
                    nc.vector.tensor_reduce(s1[:], bpp[:, 1, :], axis=AX.X, op=ALU.add)
                    nc.vector.tensor_mul(bagembT[:, 0, b:b + 1], s0[:], rz[:])
                    nc.vector.tensor_mul(bagembT[:, 1, b:b + 1], s1[:], rz[:])

        # head: out = bag_emb @ Wh.T + bh
        hdp = psum1.tile([n_bags, HEAD_DIM], F32, tag="sp")
        nc.tensor.matmul(hdp[:], bagembT[:, 0, :], wht_sb[:, 0, :], start=True, stop=False)
        nc.tensor.matmul(hdp[:], bagembT[:, 1, :], wht_sb[:, 1, :], start=False, stop=False)
        nc.tensor.matmul(hdp[:], ones_sb[:], bh2_sb[:], start=False, stop=True)
        outt = const.tile([n_bags, HEAD_DIM], F32)
        nc.scalar.copy(outt[:], hdp[:])
        nc.sync.dma_start(out_ext.ap(), outt[:])

    nc.compile()
    return nc


def prep_weights(W1, b1, Wa1, ba1, Wa2, ba2, Wh, bh, n_bags, mode="fp8"):
    f32 = np.float32
    fp8 = mode == "fp8"
    wdt = NP_F8 if fp8 else NP_BF16
    ws = W_SCALE if fp8 else 1.0
    W1 = np.asarray(W1, f32)
    Wa1 = np.asarray(Wa1, f32)
    Wa2 = np.asarray(Wa2, f32)
    Wh = np.asarray(Wh, f32)
    return {
        "w1t": (W1.T * ws).reshape(DC, P, FEAT_DIM).transpose(1, 0, 2).astype(wdt),
        "wa1t": (Wa1.T * ws).reshape(2, P, ATTN_DIM).transpose(1, 0, 2).astype(wdt),
        "wa2r": np.repeat(Wa2.reshape(P, 1), P, axis=1).astype(NP_BF16),
        "wht": np.ascontiguousarray(Wh.T.reshape(2, P, HEAD_DIM).transpose(1, 0, 2)),
        "b1c": np.ascontiguousarray(np.asarray(b1, f32).reshape(2, P).T),
        "ba1c": np.asarray(ba1, f32).reshape(P, 1).copy(),
        "bh2": np.asarray(bh, f32).reshape(1, HEAD_DIM).copy(),
        "ones_b": np.ones((1, n_bags), f32),
    }


_NC_CACHE = {}


def kernel(features, W1, b1, Wa1, ba1, Wa2, ba2, Wh, bh, bag_sizes):
    f32 = np.float32
    mode = "fp8"
    # the fp8 fast path folds b1 into a scalar activation bias, which is only
    # exact when b1 is all-zero (it is, for this model's inputs)
    if np.any(np.asarray(b1, f32) != 0.0):
        mode = "bf16"
    n_bags_core = NP_CORE // BAG

    X = np.asarray(features, f32)
    xT = X.T.astype(NP_F8 if mode == "fp8" else NP_BF16)

    shared = prep_weights(W1, b1, Wa1, ba1, Wa2, ba2, Wh, bh, n_bags_core, mode)
    in_maps = []
    for i in range(N_CORES):
        xt_i = np.ascontiguousarray(xT[:, i * NP_CORE:(i + 1) * NP_CORE])
        in_maps.append({**shared, "xt": xt_i})

    if mode not in _NC_CACHE:
        _NC_CACHE[mode] = build_nc(mode=mode)
    nc = _NC_CACHE[mode]

    res = run_bass_kernel_spmd(nc, in_maps, core_ids=list(range(N_CORES)))
    out = np.concatenate(
        [np.asarray(res.results[i]["out"], f32) for i in range(N_CORES)], axis=0
    )
    return out
